# revision 13
# baseline (speedup 1.0000x reference)
# GCGRU cell (graph-conv GRU) on 8 TRN2 NeuronCores, data-parallel over batch.
#
# Math (per batch b, N=1024 nodes, C=64 channels, Cg=192 gate width):
#   gi = adj @ (prelu(adj @ (x @ Wi0) + bi0, a_i) @ Wi1) + bi1
#   gh = adj @ (prelu(adj @ (h @ Wh0) + bh0, a_h) @ Wh1) + bh1
#   r = sigmoid(gi_r + gh_r + br); i = sigmoid(gi_i + gh_i + bi)
#   n = tanh(gi_n + r * gh_n + bn)
#   npre = (1-i)*n + i*h ; out = npre ; h' = 0.1*h + 0.9*npre
#
# Kernel restructuring (per core, BC=8 batches):
#   MM1 (ADJ):  A1 = adj @ [x | h]          -- uses associativity adj@(x@W) = (adj@x)@W
#               emitted channel-major via lhsT=[x|h], rhs=adjT  (no transposes anywhere)
#   MM2 (DEN):  S = A1 @ [Wi0|Wh0] + b0, P = prelu(S)   (channel-major, per-partition bias/alpha)
#   MM3 (DEN):  T1 = P @ W1cat   -- r/i gate sections of the i- and h-branches are summed
#               here for free via PSUM accumulation (linearity of adj@); col layout
#               [r~ , i~ , i_n , h_n] (4 x 64 = 256 cols), node-major output
#   MM4 (ADJ):  G = adj @ T1 + 1 (x) bias_row   (rank-1 matmul adds all gate biases)
#   GRU elementwise on G (node-major), h streamed from HBM.
#
# All matmul operands are bitcast to float32r (full-rate fp32 path for moving dim >= 256).

import numpy as np

import concourse.bass as bass
import concourse.bacc as bacc
import concourse.mybir as mybir
import concourse.tile as tile
from concourse.bass_utils import run_bass_kernel_spmd

B, N, TOPK, F = 64, 1024, 4, 16
C = 64
CG = 192
NCORES = 8
BC = B // NCORES
KT = N // 128   # node tiles
FP = mybir.dt.float32
FPR = mybir.dt.float32r
AF = mybir.ActivationFunctionType
OP = mybir.AluOpType


def build(prelu_native=True):
    nc = bacc.Bacc(trn_type="TRN2", debug=False, target_bir_lowering=False)

    x_d = nc.dram_tensor("x", [BC, N, C], FPR, kind="ExternalInput").ap()
    h_d = nc.dram_tensor("h", [BC, N, C], FPR, kind="ExternalInput").ap()
    hg_d = nc.dram_tensor("hg", [BC, N, C], FP, kind="ExternalInput").ap()
    xlo_d = nc.dram_tensor("xlo", [BC, N, C], FPR, kind="ExternalInput").ap()
    hlo_d = nc.dram_tensor("hlo", [BC, N, C], FPR, kind="ExternalInput").ap()
    adjt_d = nc.dram_tensor("adjt", [N, N], FPR, kind="ExternalInput").ap()
    w0_d = nc.dram_tensor("w0", [128, CG], FPR, kind="ExternalInput").ap()
    w0l_d = nc.dram_tensor("w0l", [128, CG], FPR, kind="ExternalInput").ap()
    w1a_d = nc.dram_tensor("w1a", [128, 256], FPR, kind="ExternalInput").ap()
    w1bi_d = nc.dram_tensor("w1bi", [64, 256], FPR, kind="ExternalInput").ap()
    w1bh_d = nc.dram_tensor("w1bh", [64, 256], FPR, kind="ExternalInput").ap()
    w1c_d = nc.dram_tensor("w1c", [128, 256], FPR, kind="ExternalInput").ap()
    w1al_d = nc.dram_tensor("w1al", [128, 256], FPR, kind="ExternalInput").ap()
    w1bil_d = nc.dram_tensor("w1bil", [64, 256], FPR, kind="ExternalInput").ap()
    w1bhl_d = nc.dram_tensor("w1bhl", [64, 256], FPR, kind="ExternalInput").ap()
    w1cl_d = nc.dram_tensor("w1cl", [128, 256], FPR, kind="ExternalInput").ap()
    actc_d = nc.dram_tensor("actc", [128, 16], FP, kind="ExternalInput").ap()
    grub_d = nc.dram_tensor("grub", [1, 512], FPR, kind="ExternalInput").ap()
    ones_d = nc.dram_tensor("ones", [1, 128], FPR, kind="ExternalInput").ap()
    oo_d = nc.dram_tensor("oo", [BC, N, C], FP, kind="ExternalOutput").ap()
    oh_d = nc.dram_tensor("oh", [BC, N, C], FP, kind="ExternalOutput").ap()

    # tiled views: node dim -> (tile, partition)
    x_v = x_d.rearrange("b (k p) c -> b p k c", p=128)     # [BC,128,KT,64]
    h_v = h_d.rearrange("b (k p) c -> b p k c", p=128)
    hg_v = hg_d.rearrange("b (k p) c -> b p k c", p=128)
    xlo_v = xlo_d.rearrange("b (k p) c -> b p k c", p=128)
    hlo_v = hlo_d.rearrange("b (k p) c -> b p k c", p=128)
    adjt_v = adjt_d.rearrange("(k p) n -> p k n", p=128)   # [128,KT,1024]
    oo_v = oo_d.rearrange("b (k p) c -> b p k c", p=128)
    oh_v = oh_d.rearrange("b (k p) c -> b p k c", p=128)

    with tile.TileContext(nc) as tc:
        with (
            tc.tile_pool(name="const", bufs=1) as const_pool,
            tc.tile_pool(name="xh", bufs=3) as xh_pool,
            tc.tile_pool(name="a1", bufs=2) as a1_pool,
            tc.tile_pool(name="pp", bufs=2) as p_pool,
            tc.tile_pool(name="t1", bufs=2) as t1_pool,
            tc.tile_pool(name="gru", bufs=3) as gru_pool,
            tc.tile_pool(name="ps_mm1", bufs=1, space="PSUM") as ps1_pool,
            tc.tile_pool(name="ps_mm2", bufs=1, space="PSUM") as ps2_pool,
            tc.tile_pool(name="ps_mm3", bufs=1, space="PSUM") as ps3_pool,
            tc.tile_pool(name="ps_mm4", bufs=2, space="PSUM") as ps4_pool,
        ):
            # ---- constants ----
            adjt_sb = const_pool.tile([128, KT, N], FPR)
            for kt in range(KT):
                nc.sync.dma_start(out=adjt_sb[:, kt, :], in_=adjt_v[:, kt, :])
            w0_sb = const_pool.tile([128, CG], FPR)
            nc.sync.dma_start(out=w0_sb, in_=w0_d)
            w0l_sb = const_pool.tile([128, CG], FPR)
            nc.sync.dma_start(out=w0l_sb, in_=w0l_d)
            w1a_sb = const_pool.tile([128, 256], FPR)
            nc.sync.dma_start(out=w1a_sb, in_=w1a_d)
            w1bi_sb = const_pool.tile([64, 256], FPR)
            nc.sync.dma_start(out=w1bi_sb, in_=w1bi_d)
            w1bh_sb = const_pool.tile([64, 256], FPR)
            nc.sync.dma_start(out=w1bh_sb, in_=w1bh_d)
            w1c_sb = const_pool.tile([128, 256], FPR)
            nc.sync.dma_start(out=w1c_sb, in_=w1c_d)
            w1al_sb = const_pool.tile([128, 256], FPR)
            nc.sync.dma_start(out=w1al_sb, in_=w1al_d)
            w1bil_sb = const_pool.tile([64, 256], FPR)
            nc.sync.dma_start(out=w1bil_sb, in_=w1bil_d)
            w1bhl_sb = const_pool.tile([64, 256], FPR)
            nc.sync.dma_start(out=w1bhl_sb, in_=w1bhl_d)
            w1cl_sb = const_pool.tile([128, 256], FPR)
            nc.sync.dma_start(out=w1cl_sb, in_=w1cl_d)
            actc_sb = const_pool.tile([128, 16], FP)
            nc.sync.dma_start(out=actc_sb, in_=actc_d)
            grub_sb = const_pool.tile([1, 512], FPR)
            nc.sync.dma_start(out=grub_sb, in_=grub_d)
            ones_sb = const_pool.tile([1, 128], FPR)
            nc.sync.dma_start(out=ones_sb, in_=ones_d)

            t1 = None
            for b in range(BC):
                half = b % 2

                # ---- load x|h node-major ----
                xh = xh_pool.tile([128, KT, 256], FPR)
                nc.sync.dma_start(out=xh[:, :, 0:C], in_=x_v[b])
                nc.sync.dma_start(out=xh[:, :, C:128], in_=h_v[b])
                nc.sync.dma_start(out=xh[:, :, 128:128 + C], in_=xlo_v[b])
                nc.sync.dma_start(out=xh[:, :, 128 + C:256], in_=hlo_v[b])

                # ---- MM1: A1 = [x|h].T @ adjT  (channel-major adj conv) ----
                a1 = a1_pool.tile([128, N], FPR)
                for nch in range(2):
                    ps1 = ps1_pool.tile([128, 512], FP)
                    for part in range(2):
                        for kt in range(KT):
                            nc.tensor.matmul(
                                ps1,
                                lhsT=(xh[:, kt, part * 128:(part + 1) * 128]),
                                rhs=(adjt_sb[:, kt, nch * 512:(nch + 1) * 512]),
                                start=(part == 0 and kt == 0),
                                stop=(part == 1 and kt == KT - 1),
                            )
                    nc.vector.tensor_copy(a1[:, nch * 512:(nch + 1) * 512], ps1)

                # ---- MM2: S = A1 @ [Wi0|Wh0] + b0 ; P = prelu(S) (channel-major) ----
                p1 = p_pool.tile([128, N], FPR)    # i-branch cg 0:128
                p2i = p_pool.tile([64, N], FPR)    # i-branch cg 128:192
                p2h = p_pool.tile([64, N], FPR)    # h-branch cg 128:192
                p3 = p_pool.tile([128, N], FPR)    # h-branch cg 0:128
                for nch in range(2):
                    sl = slice(nch * 512, (nch + 1) * 512)
                    psi = ps2_pool.tile([128, 512], FP)
                    psh = ps2_pool.tile([128, 512], FP)
                    psmi = ps2_pool.tile([64, 512], FP)
                    psmh = ps2_pool.tile([64, 512], FP)
                    nc.tensor.matmul(psi, lhsT=(w0_sb[0:64, 0:128]),
                                     rhs=(a1[0:64, sl]), start=True, stop=False)
                    nc.tensor.matmul(psi, lhsT=(w0l_sb[0:64, 0:128]),
                                     rhs=(a1[0:64, sl]), start=False, stop=True)
                    nc.tensor.matmul(psh, lhsT=(w0_sb[64:128, 0:128]),
                                     rhs=(a1[64:128, sl]), start=True, stop=False)
                    nc.tensor.matmul(psh, lhsT=(w0l_sb[64:128, 0:128]),
                                     rhs=(a1[64:128, sl]), start=False, stop=True)
                    nc.tensor.matmul(psmi, lhsT=(w0_sb[0:64, 128:192]),
                                     rhs=(a1[0:64, sl]), start=True, stop=False)
                    nc.tensor.matmul(psmi, lhsT=(w0l_sb[0:64, 128:192]),
                                     rhs=(a1[0:64, sl]), start=False, stop=True)
                    nc.tensor.matmul(psmh, lhsT=(w0_sb[64:128, 128:192]),
                                     rhs=(a1[64:128, sl]), start=True, stop=False)
                    nc.tensor.matmul(psmh, lhsT=(w0l_sb[64:128, 128:192]),
                                     rhs=(a1[64:128, sl]), start=False, stop=True)
                    for ps, pt, pp, ci in ((psi, p1, 128, 0), (psh, p3, 128, 1),
                                           (psmi, p2i, 64, 2), (psmh, p2h, 64, 3)):
                        if prelu_native:
                            nc.scalar.activation(
                                pt[:, sl], ps, AF.Prelu,
                                bias=actc_sb[0:pp, ci:ci + 1],
                                alpha=actc_sb[0:pp, 4 + ci:5 + ci],
                            )
                        else:
                            rl = gru_pool.tile([128, 512], FP, tag="prelu_rl")
                            av = gru_pool.tile([128, 512], FP, tag="prelu_av")
                            nc.scalar.activation(rl[0:pp, :], ps, AF.Relu,
                                                 bias=actc_sb[0:pp, ci:ci + 1])
                            nc.scalar.activation(av[0:pp, :], ps, AF.Identity,
                                                 bias=actc_sb[0:pp, 8 + ci:9 + ci],
                                                 scale=actc_sb[0:pp, 4 + ci:5 + ci])
                            nc.vector.scalar_tensor_tensor(
                                pt[:, sl], rl[0:pp, :], actc_sb[0:pp, 12 + ci:13 + ci],
                                av[0:pp, :], op0=OP.mult, op1=OP.add)

                # ---- MM3: T1 = P @ W1cat (node-major, gate-merged 256 cols) ----
                if half == 0:
                    t1 = t1_pool.tile([128, KT, 512], FPR)
                for nblk in range(KT):
                    nsl = slice(nblk * 128, (nblk + 1) * 128)
                    ps3 = ps3_pool.tile([128, 256], FP)
                    nc.tensor.matmul(ps3, lhsT=(p1[:, nsl]), rhs=(w1a_sb),
                                     start=True, stop=False)
                    nc.tensor.matmul(ps3, lhsT=(p1[:, nsl]), rhs=(w1al_sb),
                                     start=False, stop=False)
                    nc.tensor.matmul(ps3, lhsT=(p2i[:, nsl]), rhs=(w1bi_sb),
                                     start=False, stop=False)
                    nc.tensor.matmul(ps3, lhsT=(p2i[:, nsl]), rhs=(w1bil_sb),
                                     start=False, stop=False)
                    nc.tensor.matmul(ps3, lhsT=(p3[:, nsl]), rhs=(w1c_sb),
                                     start=False, stop=False)
                    nc.tensor.matmul(ps3, lhsT=(p3[:, nsl]), rhs=(w1cl_sb),
                                     start=False, stop=False)
                    nc.tensor.matmul(ps3, lhsT=(p2h[:, nsl]), rhs=(w1bh_sb),
                                     start=False, stop=False)
                    nc.tensor.matmul(ps3, lhsT=(p2h[:, nsl]), rhs=(w1bhl_sb),
                                     start=False, stop=True)
                    nc.scalar.copy(t1[:, nblk, half * 256:(half + 1) * 256], ps3)

                # ---- MM4 + GRU for the completed pair ----
                if half == 1:
                    b0 = b - 1
                    hx_all = gru_pool.tile([128, KT, 2, C], FP)
                    nc.sync.dma_start(out=hx_all[:, :, 0, :], in_=hg_v[b0])
                    nc.sync.dma_start(out=hx_all[:, :, 1, :], in_=hg_v[b])
                    oo_all = gru_pool.tile([128, KT, 2, C], FP)
                    oh_all = gru_pool.tile([128, KT, 2, C], FP)
                    for nblk in range(KT):
                        ps4 = ps4_pool.tile([128, 512], FP)
                        nc.tensor.matmul(ps4, lhsT=(ones_sb), rhs=(grub_sb),
                                         start=True, stop=False)
                        for kt in range(KT):
                            nc.tensor.matmul(
                                ps4,
                                lhsT=(adjt_sb[:, kt, nblk * 128:(nblk + 1) * 128]),
                                rhs=(t1[:, kt, :]),
                                start=False,
                                stop=(kt == KT - 1),
                            )
                        # ps4 cols per batch: [r~ 0:64 | i~ 64:128 | i_n 128:192 | h_n 192:256]
                        g = ps4.rearrange("p (two c) -> p two c", two=2)
                        ri = gru_pool.tile([128, 2, 128], FP)
                        nc.scalar.activation(ri, g[:, :, 0:128], AF.Sigmoid)
                        hx = hx_all[:, nblk]
                        t_ = gru_pool.tile([128, 2, C], FP)
                        nc.vector.tensor_mul(t_, ri[:, :, 0:64], g[:, :, 192:256])
                        u = gru_pool.tile([128, 2, C], FP)
                        nc.vector.tensor_add(u, t_, g[:, :, 128:192])
                        ng = gru_pool.tile([128, 2, C], FP)
                        nc.scalar.activation(ng, u, AF.Tanh)
                        d = gru_pool.tile([128, 2, C], FP)
                        nc.vector.tensor_sub(d, hx, ng)
                        e = gru_pool.tile([128, 2, C], FP)
                        nc.vector.tensor_mul(e, d, ri[:, :, 64:128])
                        npre = oo_all[:, nblk]
                        nc.vector.tensor_add(npre, e, ng)
                        d2 = gru_pool.tile([128, 2, C], FP)
                        nc.vector.tensor_sub(d2, hx, npre)
                        nh = oh_all[:, nblk]
                        nc.vector.scalar_tensor_tensor(nh, d2, 0.1, npre,
                                                       op0=OP.mult, op1=OP.add)
                    for i_, bb in ((0, b0), (1, b)):
                        nc.sync.dma_start(out=oo_v[bb], in_=oo_all[:, :, i_, :])
                        nc.sync.dma_start(out=oh_v[bb], in_=oh_all[:, :, i_, :])
    nc.compile()
    return nc


def round_fp32r(a):
    """Round fp32 to the PE's fp32r format: RNE to 11 mantissa bits."""
    u = np.ascontiguousarray(a, dtype=np.float32).view(np.uint32)
    r = u + (((u >> np.uint32(12)) & np.uint32(1)) + np.uint32(0x7FF))
    r &= np.uint32(0xFFFFF000)
    return r.view(np.float32)


def host_prep(inputs, adj, hidden, Wi0, bi0, Wi1, bi1, a_i,
              Wh0, bh0, Wh1, bh1, a_h, bias_r, bias_i, bias_n):
    f32 = lambda a: np.ascontiguousarray(np.asarray(a), dtype=np.float32)
    x = f32(inputs).reshape(B, N, C)
    h = f32(hidden)
    adjt = np.ascontiguousarray(f32(adj).T)
    Wi0, Wh0, Wi1, Wh1 = f32(Wi0), f32(Wh0), f32(Wi1), f32(Wh1)
    bi0, bh0, bi1, bh1 = f32(bi0), f32(bh0), f32(bi1), f32(bh1)
    bias_r, bias_i, bias_n = f32(bias_r), f32(bias_i), f32(bias_n)
    a_i = float(np.asarray(a_i)); a_h = float(np.asarray(a_h))

    w0 = np.concatenate([Wi0, Wh0], axis=0)                     # [128,192]
    z64 = np.zeros((128, 64), np.float32)
    w1a = np.concatenate([Wi1[0:128, 0:64], Wi1[0:128, 64:128],
                          Wi1[0:128, 128:192], z64], axis=1)    # i rows 0:128
    w1c = np.concatenate([Wh1[0:128, 0:64], Wh1[0:128, 64:128],
                          z64, Wh1[0:128, 128:192]], axis=1)    # h rows 0:128
    z = np.zeros((64, 64), np.float32)
    w1bi = np.concatenate([Wi1[128:192, 0:64], Wi1[128:192, 64:128],
                           Wi1[128:192, 128:192], z], axis=1)   # [64,256]
    w1bh = np.concatenate([Wh1[128:192, 0:64], Wh1[128:192, 64:128],
                           z, Wh1[128:192, 128:192]], axis=1)   # [64,256]

    pad64 = lambda v: np.concatenate([v, np.zeros(64, np.float32)])
    bias_cols = np.stack([bi0[0:128], bh0[0:128],
                          pad64(bi0[128:192]), pad64(bh0[128:192])], axis=1)
    alpha_cols = np.stack([np.full(128, a_i, np.float32),
                           np.full(128, a_h, np.float32),
                           np.full(128, a_i, np.float32),
                           np.full(128, a_h, np.float32)], axis=1)
    actc = np.concatenate([bias_cols, alpha_cols,
                           alpha_cols * bias_cols, 1.0 - alpha_cols],
                          axis=1).astype(np.float32)            # [128,16]

    gb = np.concatenate([bi1[0:64] + bh1[0:64] + bias_r,
                         bi1[64:128] + bh1[64:128] + bias_i,
                         bi1[128:192] + bias_n,
                         bh1[128:192]])
    grub = np.tile(gb, 2)[None, :].astype(np.float32)           # [1,512]

    def split_r(a):
        hi = round_fp32r(a)
        lo = round_fp32r(a - hi)
        return hi, lo

    w0_h, w0_l = split_r(w0)
    w1a_h, w1a_l = split_r(w1a)
    w1bi_h, w1bi_l = split_r(w1bi)
    w1bh_h, w1bh_l = split_r(w1bh)
    w1c_h, w1c_l = split_r(w1c)
    shared = dict(adjt=round_fp32r(adjt), w0=w0_h, w0l=w0_l,
                  w1a=w1a_h, w1al=w1a_l, w1bi=w1bi_h, w1bil=w1bi_l,
                  w1bh=w1bh_h, w1bhl=w1bh_l,
                  w1c=w1c_h, w1cl=w1c_l, actc=actc, grub=round_fp32r(grub),
                  ones=np.ones((1, 128), np.float32))
    xr, xlo = split_r(x)
    hr, hlo = split_r(h)
    in_maps = []
    for i in range(NCORES):
        m = dict(shared)
        sl = slice(i * BC, (i + 1) * BC)
        m["x"] = np.ascontiguousarray(xr[sl])
        m["h"] = np.ascontiguousarray(hr[sl])
        m["xlo"] = np.ascontiguousarray(xlo[sl])
        m["hlo"] = np.ascontiguousarray(hlo[sl])
        m["hg"] = np.ascontiguousarray(h[sl])
        in_maps.append(m)
    return in_maps


_CACHE = {}


def get_nc(prelu_native=True):
    key = prelu_native
    if key not in _CACHE:
        _CACHE[key] = build(prelu_native=prelu_native)
    return _CACHE[key]


def run(in_maps, trace=False, prelu_native=True, **kw):
    nc = get_nc(prelu_native)
    return run_bass_kernel_spmd(nc, in_maps, core_ids=list(range(NCORES)),
                                trace=trace, **kw)


def assemble(results):
    oo = np.concatenate([r["oo"] for r in results], axis=0)
    oh = np.concatenate([r["oh"] for r in results], axis=0)
    return oo.reshape(B, N, TOPK, F), oh


def kernel(**inputs):
    in_maps = host_prep(**inputs)
    res = run(in_maps, trace=False)
    return assemble(res.results)


# revision 20
# speedup vs baseline: 65.8586x; 65.8586x over previous
# GCGRU cell (graph-conv GRU) on 8 TRN2 NeuronCores, data-parallel over batch.
#
# Math (per batch b, N=1024 nodes, C=64 channels, Cg=192 gate width):
#   gi = adj @ (prelu(adj @ (x @ Wi0) + bi0, a_i) @ Wi1) + bi1
#   gh = adj @ (prelu(adj @ (h @ Wh0) + bh0, a_h) @ Wh1) + bh1
#   r = sigmoid(gi_r + gh_r + br); i = sigmoid(gi_i + gh_i + bi)
#   n = tanh(gi_n + r * gh_n + bn)
#   npre = (1-i)*n + i*h ; out = npre ; h' = 0.1*h + 0.9*npre
#
# Kernel restructuring (per core, BC=8 batches):
#   MM1 (ADJ):  A1 = adj @ [x | h]          -- uses associativity adj@(x@W) = (adj@x)@W
#               emitted channel-major via lhsT=[x|h], rhs=adjT  (no transposes anywhere)
#   MM2 (DEN):  S = A1 @ [Wi0|Wh0] + b0, P = prelu(S)   (channel-major, per-partition bias/alpha)
#   MM3 (DEN):  T1 = P @ W1cat   -- r/i gate sections of the i- and h-branches are summed
#               here for free via PSUM accumulation (linearity of adj@); col layout
#               [r~ , i~ , i_n , h_n] (4 x 64 = 256 cols), node-major output
#   MM4 (ADJ):  G = adj @ T1 + 1 (x) bias_row   (rank-1 matmul adds all gate biases)
#   GRU elementwise on G (node-major), h streamed from HBM.
#
# All matmul operands are bitcast to float32r (full-rate fp32 path for moving dim >= 256).

import numpy as np

import concourse.bass as bass
import concourse.bacc as bacc
import concourse.mybir as mybir
import concourse.tile as tile
from concourse.bass_utils import run_bass_kernel_spmd

B, N, TOPK, F = 64, 1024, 4, 16
C = 64
CG = 192
NCORES = 8
BC = B // NCORES
KT = N // 128   # node tiles
FP = mybir.dt.float32
FPR = mybir.dt.float32r
AF = mybir.ActivationFunctionType
OP = mybir.AluOpType


def build(prelu_native=True):
    nc = bacc.Bacc(trn_type="TRN2", debug=False, target_bir_lowering=False)

    x_d = nc.dram_tensor("x", [BC, N, C], FPR, kind="ExternalInput").ap()
    h_d = nc.dram_tensor("h", [BC, N, C], FPR, kind="ExternalInput").ap()
    hg_d = nc.dram_tensor("hg", [BC, N, C], FP, kind="ExternalInput").ap()
    xlo_d = nc.dram_tensor("xlo", [BC, N, C], FPR, kind="ExternalInput").ap()
    hlo_d = nc.dram_tensor("hlo", [BC, N, C], FPR, kind="ExternalInput").ap()
    adjt_d = nc.dram_tensor("adjt", [N, N], FPR, kind="ExternalInput").ap()
    w0_d = nc.dram_tensor("w0", [128, CG], FPR, kind="ExternalInput").ap()
    w0l_d = nc.dram_tensor("w0l", [128, CG], FPR, kind="ExternalInput").ap()
    w1a_d = nc.dram_tensor("w1a", [128, 256], FPR, kind="ExternalInput").ap()
    w1bi_d = nc.dram_tensor("w1bi", [64, 256], FPR, kind="ExternalInput").ap()
    w1bh_d = nc.dram_tensor("w1bh", [64, 256], FPR, kind="ExternalInput").ap()
    w1c_d = nc.dram_tensor("w1c", [128, 256], FPR, kind="ExternalInput").ap()
    w1al_d = nc.dram_tensor("w1al", [128, 256], FPR, kind="ExternalInput").ap()
    w1bil_d = nc.dram_tensor("w1bil", [64, 256], FPR, kind="ExternalInput").ap()
    w1bhl_d = nc.dram_tensor("w1bhl", [64, 256], FPR, kind="ExternalInput").ap()
    w1cl_d = nc.dram_tensor("w1cl", [128, 256], FPR, kind="ExternalInput").ap()
    actc_d = nc.dram_tensor("actc", [128, 16], FP, kind="ExternalInput").ap()
    grub_d = nc.dram_tensor("grub", [1, 512], FPR, kind="ExternalInput").ap()
    ones_d = nc.dram_tensor("ones", [1, 128], FPR, kind="ExternalInput").ap()
    oo_d = nc.dram_tensor("oo", [BC, N, C], FP, kind="ExternalOutput").ap()
    oh_d = nc.dram_tensor("oh", [BC, N, C], FP, kind="ExternalOutput").ap()

    # tiled views: node dim -> (tile, partition)
    x_v = x_d.rearrange("b (k p) c -> b p k c", p=128)     # [BC,128,KT,64]
    h_v = h_d.rearrange("b (k p) c -> b p k c", p=128)
    hg_v = hg_d.rearrange("b (k p) c -> b p k c", p=128)
    xlo_v = xlo_d.rearrange("b (k p) c -> b p k c", p=128)
    hlo_v = hlo_d.rearrange("b (k p) c -> b p k c", p=128)
    adjt_v = adjt_d.rearrange("(k p) n -> p k n", p=128)   # [128,KT,1024]
    oo_v = oo_d.rearrange("b (k p) c -> b p k c", p=128)
    oh_v = oh_d.rearrange("b (k p) c -> b p k c", p=128)

    with tile.TileContext(nc) as tc:
        with (
            tc.tile_pool(name="const", bufs=1) as const_pool,
            tc.tile_pool(name="xh", bufs=3) as xh_pool,
            tc.tile_pool(name="a1", bufs=2) as a1_pool,
            tc.tile_pool(name="pp", bufs=2) as p_pool,
            tc.tile_pool(name="t1", bufs=2) as t1_pool,
            tc.tile_pool(name="gru", bufs=2) as gru_pool,
            tc.tile_pool(name="ps_mm1", bufs=1, space="PSUM") as ps1_pool,
            tc.tile_pool(name="ps_mm2", bufs=1, space="PSUM") as ps2_pool,
            tc.tile_pool(name="ps_mm3", bufs=2, space="PSUM") as ps3_pool,
            tc.tile_pool(name="ps_mm4", bufs=2, space="PSUM") as ps4_pool,
        ):
            # ---- constants ----
            adjt_sb = const_pool.tile([128, KT, N], FPR)
            for kt in range(KT):
                nc.sync.dma_start(out=adjt_sb[:, kt, :], in_=adjt_v[:, kt, :])
            w0_sb = const_pool.tile([128, CG], FPR)
            nc.sync.dma_start(out=w0_sb, in_=w0_d)
            w0l_sb = const_pool.tile([128, CG], FPR)
            nc.sync.dma_start(out=w0l_sb, in_=w0l_d)
            w1a_sb = const_pool.tile([128, 256], FPR)
            nc.sync.dma_start(out=w1a_sb, in_=w1a_d)
            w1bi_sb = const_pool.tile([64, 256], FPR)
            nc.sync.dma_start(out=w1bi_sb, in_=w1bi_d)
            w1bh_sb = const_pool.tile([64, 256], FPR)
            nc.sync.dma_start(out=w1bh_sb, in_=w1bh_d)
            w1c_sb = const_pool.tile([128, 256], FPR)
            nc.sync.dma_start(out=w1c_sb, in_=w1c_d)
            w1al_sb = const_pool.tile([128, 256], FPR)
            nc.sync.dma_start(out=w1al_sb, in_=w1al_d)
            w1bil_sb = const_pool.tile([64, 256], FPR)
            nc.sync.dma_start(out=w1bil_sb, in_=w1bil_d)
            w1bhl_sb = const_pool.tile([64, 256], FPR)
            nc.sync.dma_start(out=w1bhl_sb, in_=w1bhl_d)
            w1cl_sb = const_pool.tile([128, 256], FPR)
            nc.sync.dma_start(out=w1cl_sb, in_=w1cl_d)
            actc_sb = const_pool.tile([128, 16], FP)
            nc.sync.dma_start(out=actc_sb, in_=actc_d)
            grub_sb = const_pool.tile([1, 512], FPR)
            nc.sync.dma_start(out=grub_sb, in_=grub_d)
            ones_sb = const_pool.tile([1, 128], FPR)
            nc.sync.dma_start(out=ones_sb, in_=ones_d)

            t1 = None
            t1_of_pair = {}

            def do_mm4(pair):
                t1p = t1_of_pair.pop(pair)
                b0, b1 = 2 * pair, 2 * pair + 1
                hx_all = gru_pool.tile([128, KT, 2, C], FP, name=f"hx_all_{pair}", tag="hx_all")
                nc.sync.dma_start(out=hx_all[:, :, 0, :], in_=hg_v[b0])
                nc.sync.dma_start(out=hx_all[:, :, 1, :], in_=hg_v[b1])
                oo_all = gru_pool.tile([128, KT, 2, C], FP, name=f"oo_all_{pair}", tag="oo_all")
                oh_all = gru_pool.tile([128, KT, 2, C], FP, name=f"oh_all_{pair}", tag="oh_all")
                for nblk in range(KT):
                    ps4 = ps4_pool.tile([128, 512], FP, name=f"ps4_{pair}_{nblk}", tag="ps4")
                    nc.tensor.matmul(ps4, lhsT=(ones_sb), rhs=(grub_sb),
                                     start=True, stop=False)
                    for kt in range(KT):
                        nc.tensor.matmul(
                            ps4,
                            lhsT=(adjt_sb[:, kt, nblk * 128:(nblk + 1) * 128]),
                            rhs=(t1p[:, kt, :]),
                            start=False,
                            stop=(kt == KT - 1),
                        )
                    # ps4 cols per batch: [r~ 0:64 | i~ 64:128 | i_n 128:192 | h_n 192:256]
                    g = ps4.rearrange("p (two c) -> p two c", two=2)
                    ri = gru_pool.tile([128, 2, 128], FP, name=f"ri_{pair}_{nblk}", tag="ri")
                    nc.scalar.activation(ri, g[:, :, 0:128], AF.Sigmoid)
                    hx = hx_all[:, nblk]
                    t_ = gru_pool.tile([128, 2, C], FP, name=f"t_{pair}_{nblk}", tag="t_")
                    nc.vector.tensor_mul(t_, ri[:, :, 0:64], g[:, :, 192:256])
                    u = gru_pool.tile([128, 2, C], FP, name=f"u_{pair}_{nblk}", tag="u")
                    nc.vector.tensor_add(u, t_, g[:, :, 128:192])
                    ng = gru_pool.tile([128, 2, C], FP, name=f"ng_{pair}_{nblk}", tag="ng")
                    nc.scalar.activation(ng, u, AF.Tanh)
                    d = gru_pool.tile([128, 2, C], FP, name=f"d_{pair}_{nblk}", tag="d")
                    nc.vector.tensor_sub(d, hx, ng)
                    e = gru_pool.tile([128, 2, C], FP, name=f"e_{pair}_{nblk}", tag="e")
                    nc.vector.tensor_mul(e, d, ri[:, :, 64:128])
                    npre = oo_all[:, nblk]
                    nc.vector.tensor_add(npre, e, ng)
                    d2 = gru_pool.tile([128, 2, C], FP, name=f"d2_{pair}_{nblk}", tag="d2")
                    nc.vector.tensor_sub(d2, hx, npre)
                    nh = oh_all[:, nblk]
                    nc.vector.scalar_tensor_tensor(nh, d2, 0.1, npre,
                                                   op0=OP.mult, op1=OP.add)
                for i_, bb in ((0, b0), (1, b1)):
                    nc.sync.dma_start(out=oo_v[bb], in_=oo_all[:, :, i_, :])
                    nc.sync.dma_start(out=oh_v[bb], in_=oh_all[:, :, i_, :])

            for b in range(BC):
                half = b % 2
                if half == 0 and b >= 2:
                    do_mm4(b // 2 - 1)

                # ---- load x|h node-major ----
                xh = xh_pool.tile([128, KT, 256], FPR)
                nc.sync.dma_start(out=xh[:, :, 0:C], in_=x_v[b])
                nc.sync.dma_start(out=xh[:, :, C:128], in_=h_v[b])
                nc.sync.dma_start(out=xh[:, :, 128:128 + C], in_=xlo_v[b])
                nc.sync.dma_start(out=xh[:, :, 128 + C:256], in_=hlo_v[b])

                # ---- MM1: A1 = [x|h].T @ adjT  (channel-major adj conv) ----
                a1 = a1_pool.tile([128, N], FPR)
                for nch in range(2):
                    ps1 = ps1_pool.tile([128, 512], FP, name="ps1")
                    for part in range(2):
                        for kt in range(KT):
                            nc.tensor.matmul(
                                ps1,
                                lhsT=(xh[:, kt, part * 128:(part + 1) * 128]),
                                rhs=(adjt_sb[:, kt, nch * 512:(nch + 1) * 512]),
                                start=(part == 0 and kt == 0),
                                stop=(part == 1 and kt == KT - 1),
                            )
                    nc.vector.tensor_copy(a1[:, nch * 512:(nch + 1) * 512], ps1)

                # ---- MM2: S = A1 @ [Wi0|Wh0] + b0 ; P = prelu(S) (channel-major) ----
                p1 = p_pool.tile([128, N], FPR)    # i-branch cg 0:128
                p2i = p_pool.tile([64, N], FPR)    # i-branch cg 128:192
                p2h = p_pool.tile([64, N], FPR)    # h-branch cg 128:192
                p3 = p_pool.tile([128, N], FPR)    # h-branch cg 0:128
                for nch in range(2):
                    sl = slice(nch * 512, (nch + 1) * 512)
                    psi = ps2_pool.tile([128, 512], FP)
                    psh = ps2_pool.tile([128, 512], FP)
                    psmi = ps2_pool.tile([64, 512], FP, tag="psm", name="psmi")
                    psmh = ps2_pool.tile([64, 512], FP, tag="psm", name="psmh")
                    nc.tensor.matmul(psi, lhsT=(w0_sb[0:64, 0:128]),
                                     rhs=(a1[0:64, sl]), start=True, stop=False)
                    nc.tensor.matmul(psi, lhsT=(w0l_sb[0:64, 0:128]),
                                     rhs=(a1[0:64, sl]), start=False, stop=True)
                    nc.tensor.matmul(psh, lhsT=(w0_sb[64:128, 0:128]),
                                     rhs=(a1[64:128, sl]), start=True, stop=False)
                    nc.tensor.matmul(psh, lhsT=(w0l_sb[64:128, 0:128]),
                                     rhs=(a1[64:128, sl]), start=False, stop=True)
                    nc.tensor.matmul(psmi, lhsT=(w0_sb[0:64, 128:192]),
                                     rhs=(a1[0:64, sl]), start=True, stop=False)
                    nc.tensor.matmul(psmi, lhsT=(w0l_sb[0:64, 128:192]),
                                     rhs=(a1[0:64, sl]), start=False, stop=True)
                    nc.tensor.matmul(psmh, lhsT=(w0_sb[64:128, 128:192]),
                                     rhs=(a1[64:128, sl]), start=True, stop=False)
                    nc.tensor.matmul(psmh, lhsT=(w0l_sb[64:128, 128:192]),
                                     rhs=(a1[64:128, sl]), start=False, stop=True)
                    for ps, pt, pp, ci in ((psi, p1, 128, 0), (psh, p3, 128, 1),
                                           (psmi, p2i, 64, 2), (psmh, p2h, 64, 3)):
                        if prelu_native:
                            nc.scalar.activation(
                                pt[:, sl], ps, AF.Prelu,
                                bias=actc_sb[0:pp, ci:ci + 1],
                                alpha=actc_sb[0:pp, 4 + ci:5 + ci],
                            )
                        else:
                            rl = gru_pool.tile([128, 512], FP, tag="prelu_rl")
                            av = gru_pool.tile([128, 512], FP, tag="prelu_av")
                            nc.scalar.activation(rl[0:pp, :], ps, AF.Relu,
                                                 bias=actc_sb[0:pp, ci:ci + 1])
                            nc.scalar.activation(av[0:pp, :], ps, AF.Identity,
                                                 bias=actc_sb[0:pp, 8 + ci:9 + ci],
                                                 scale=actc_sb[0:pp, 4 + ci:5 + ci])
                            nc.vector.scalar_tensor_tensor(
                                pt[:, sl], rl[0:pp, :], actc_sb[0:pp, 12 + ci:13 + ci],
                                av[0:pp, :], op0=OP.mult, op1=OP.add)

                # ---- MM3: T1 = P @ W1cat (node-major, gate-merged 256 cols) ----
                if half == 0:
                    t1 = t1_pool.tile([128, KT, 512], FPR, name=f"t1_{b // 2}", tag="t1")
                    t1_of_pair[b // 2] = t1
                for nblk in range(KT):
                    nsl = slice(nblk * 128, (nblk + 1) * 128)
                    ps3 = ps3_pool.tile([128, 256], FP, name="ps3")
                    nc.tensor.matmul(ps3, lhsT=(p1[:, nsl]), rhs=(w1a_sb),
                                     start=True, stop=False)
                    nc.tensor.matmul(ps3, lhsT=(p1[:, nsl]), rhs=(w1al_sb),
                                     start=False, stop=False)
                    nc.tensor.matmul(ps3, lhsT=(p2i[:, nsl]), rhs=(w1bi_sb),
                                     start=False, stop=False)
                    nc.tensor.matmul(ps3, lhsT=(p2i[:, nsl]), rhs=(w1bil_sb),
                                     start=False, stop=False)
                    nc.tensor.matmul(ps3, lhsT=(p3[:, nsl]), rhs=(w1c_sb),
                                     start=False, stop=False)
                    nc.tensor.matmul(ps3, lhsT=(p3[:, nsl]), rhs=(w1cl_sb),
                                     start=False, stop=False)
                    nc.tensor.matmul(ps3, lhsT=(p2h[:, nsl]), rhs=(w1bh_sb),
                                     start=False, stop=False)
                    nc.tensor.matmul(ps3, lhsT=(p2h[:, nsl]), rhs=(w1bhl_sb),
                                     start=False, stop=True)
                    nc.scalar.copy(t1[:, nblk, half * 256:(half + 1) * 256], ps3)

            do_mm4(BC // 2 - 1)

    nc.compile()
    return nc


def round_fp32r(a):
    """Round fp32 to the PE's fp32r format: RNE to 11 mantissa bits."""
    u = np.ascontiguousarray(a, dtype=np.float32).view(np.uint32)
    r = u + (((u >> np.uint32(12)) & np.uint32(1)) + np.uint32(0x7FF))
    r &= np.uint32(0xFFFFF000)
    return r.view(np.float32)


def host_prep(inputs, adj, hidden, Wi0, bi0, Wi1, bi1, a_i,
              Wh0, bh0, Wh1, bh1, a_h, bias_r, bias_i, bias_n):
    f32 = lambda a: np.ascontiguousarray(np.asarray(a), dtype=np.float32)
    x = f32(inputs).reshape(B, N, C)
    h = f32(hidden)
    adjt = np.ascontiguousarray(f32(adj).T)
    Wi0, Wh0, Wi1, Wh1 = f32(Wi0), f32(Wh0), f32(Wi1), f32(Wh1)
    bi0, bh0, bi1, bh1 = f32(bi0), f32(bh0), f32(bi1), f32(bh1)
    bias_r, bias_i, bias_n = f32(bias_r), f32(bias_i), f32(bias_n)
    a_i = float(np.asarray(a_i)); a_h = float(np.asarray(a_h))

    w0 = np.concatenate([Wi0, Wh0], axis=0)                     # [128,192]
    z64 = np.zeros((128, 64), np.float32)
    w1a = np.concatenate([Wi1[0:128, 0:64], Wi1[0:128, 64:128],
                          Wi1[0:128, 128:192], z64], axis=1)    # i rows 0:128
    w1c = np.concatenate([Wh1[0:128, 0:64], Wh1[0:128, 64:128],
                          z64, Wh1[0:128, 128:192]], axis=1)    # h rows 0:128
    z = np.zeros((64, 64), np.float32)
    w1bi = np.concatenate([Wi1[128:192, 0:64], Wi1[128:192, 64:128],
                           Wi1[128:192, 128:192], z], axis=1)   # [64,256]
    w1bh = np.concatenate([Wh1[128:192, 0:64], Wh1[128:192, 64:128],
                           z, Wh1[128:192, 128:192]], axis=1)   # [64,256]

    pad64 = lambda v: np.concatenate([v, np.zeros(64, np.float32)])
    bias_cols = np.stack([bi0[0:128], bh0[0:128],
                          pad64(bi0[128:192]), pad64(bh0[128:192])], axis=1)
    alpha_cols = np.stack([np.full(128, a_i, np.float32),
                           np.full(128, a_h, np.float32),
                           np.full(128, a_i, np.float32),
                           np.full(128, a_h, np.float32)], axis=1)
    actc = np.concatenate([bias_cols, alpha_cols,
                           alpha_cols * bias_cols, 1.0 - alpha_cols],
                          axis=1).astype(np.float32)            # [128,16]

    gb = np.concatenate([bi1[0:64] + bh1[0:64] + bias_r,
                         bi1[64:128] + bh1[64:128] + bias_i,
                         bi1[128:192] + bias_n,
                         bh1[128:192]])
    grub = np.tile(gb, 2)[None, :].astype(np.float32)           # [1,512]

    def split_r(a):
        hi = round_fp32r(a)
        lo = round_fp32r(a - hi)
        return hi, lo

    w0_h, w0_l = split_r(w0)
    w1a_h, w1a_l = split_r(w1a)
    w1bi_h, w1bi_l = split_r(w1bi)
    w1bh_h, w1bh_l = split_r(w1bh)
    w1c_h, w1c_l = split_r(w1c)
    shared = dict(adjt=round_fp32r(adjt), w0=w0_h, w0l=w0_l,
                  w1a=w1a_h, w1al=w1a_l, w1bi=w1bi_h, w1bil=w1bi_l,
                  w1bh=w1bh_h, w1bhl=w1bh_l,
                  w1c=w1c_h, w1cl=w1c_l, actc=actc, grub=round_fp32r(grub),
                  ones=np.ones((1, 128), np.float32))
    xr, xlo = split_r(x)
    hr, hlo = split_r(h)
    in_maps = []
    for i in range(NCORES):
        m = dict(shared)
        sl = slice(i * BC, (i + 1) * BC)
        m["x"] = np.ascontiguousarray(xr[sl])
        m["h"] = np.ascontiguousarray(hr[sl])
        m["xlo"] = np.ascontiguousarray(xlo[sl])
        m["hlo"] = np.ascontiguousarray(hlo[sl])
        m["hg"] = np.ascontiguousarray(h[sl])
        in_maps.append(m)
    return in_maps


_CACHE = {}


def get_nc(prelu_native=True):
    key = prelu_native
    if key not in _CACHE:
        _CACHE[key] = build(prelu_native=prelu_native)
    return _CACHE[key]


def run(in_maps, trace=False, prelu_native=True, **kw):
    nc = get_nc(prelu_native)
    return run_bass_kernel_spmd(nc, in_maps, core_ids=list(range(NCORES)),
                                trace=trace, **kw)


def assemble(results):
    oo = np.concatenate([r["oo"] for r in results], axis=0)
    oh = np.concatenate([r["oh"] for r in results], axis=0)
    return oo.reshape(B, N, TOPK, F), oh


def kernel(**inputs):
    in_maps = host_prep(**inputs)
    res = run(in_maps, trace=False)
    return assemble(res.results)


# revision 22
# speedup vs baseline: 331.6190x; 5.0353x over previous
# GCGRU cell (graph-conv GRU) on 8 TRN2 NeuronCores, data-parallel over batch.
#
# Math (per batch b, N=1024 nodes, C=64 channels, Cg=192 gate width):
#   gi = adj @ (prelu(adj @ (x @ Wi0) + bi0, a_i) @ Wi1) + bi1
#   gh = adj @ (prelu(adj @ (h @ Wh0) + bh0, a_h) @ Wh1) + bh1
#   r = sigmoid(gi_r + gh_r + br); i = sigmoid(gi_i + gh_i + bi)
#   n = tanh(gi_n + r * gh_n + bn)
#   npre = (1-i)*n + i*h ; out = npre ; h' = 0.1*h + 0.9*npre
#
# Kernel restructuring (per core, BC=8 batches):
#   MM1 (ADJ):  A1 = adj @ [x | h]          -- uses associativity adj@(x@W) = (adj@x)@W
#               emitted channel-major via lhsT=[x|h], rhs=adjT  (no transposes anywhere)
#   MM2 (DEN):  S = A1 @ [Wi0|Wh0] + b0, P = prelu(S)   (channel-major, per-partition bias/alpha)
#   MM3 (DEN):  T1 = P @ W1cat   -- r/i gate sections of the i- and h-branches are summed
#               here for free via PSUM accumulation (linearity of adj@); col layout
#               [r~ , i~ , i_n , h_n] (4 x 64 = 256 cols), node-major output
#   MM4 (ADJ):  G = adj @ T1 + 1 (x) bias_row   (rank-1 matmul adds all gate biases)
#   GRU elementwise on G (node-major), h streamed from HBM.
#
# All matmul operands are bitcast to float32r (full-rate fp32 path for moving dim >= 256).

import numpy as np

import concourse.bass as bass
import concourse.bacc as bacc
import concourse.mybir as mybir
import concourse.tile as tile
from concourse.bass_utils import run_bass_kernel_spmd

B, N, TOPK, F = 64, 1024, 4, 16
C = 64
CG = 192
NCORES = 8
BC = B // NCORES
KT = N // 128   # node tiles
FP = mybir.dt.float32
FPR = mybir.dt.float32r
AF = mybir.ActivationFunctionType
OP = mybir.AluOpType


def build(prelu_native=True):
    nc = bacc.Bacc(trn_type="TRN2", debug=False, target_bir_lowering=False)

    x_d = nc.dram_tensor("x", [BC, N, C], FPR, kind="ExternalInput").ap()
    h_d = nc.dram_tensor("h", [BC, N, C], FPR, kind="ExternalInput").ap()
    hg_d = nc.dram_tensor("hg", [BC, N, C], FP, kind="ExternalInput").ap()
    xlo_d = nc.dram_tensor("xlo", [BC, N, C], FPR, kind="ExternalInput").ap()
    hlo_d = nc.dram_tensor("hlo", [BC, N, C], FPR, kind="ExternalInput").ap()
    adjt_d = nc.dram_tensor("adjt", [N, N], FPR, kind="ExternalInput").ap()
    w0_d = nc.dram_tensor("w0", [128, CG], FPR, kind="ExternalInput").ap()
    w0l_d = nc.dram_tensor("w0l", [128, CG], FPR, kind="ExternalInput").ap()
    w1a_d = nc.dram_tensor("w1a", [128, 256], FPR, kind="ExternalInput").ap()
    w1bi_d = nc.dram_tensor("w1bi", [64, 256], FPR, kind="ExternalInput").ap()
    w1bh_d = nc.dram_tensor("w1bh", [64, 256], FPR, kind="ExternalInput").ap()
    w1c_d = nc.dram_tensor("w1c", [128, 256], FPR, kind="ExternalInput").ap()
    w1al_d = nc.dram_tensor("w1al", [128, 256], FPR, kind="ExternalInput").ap()
    w1bil_d = nc.dram_tensor("w1bil", [64, 256], FPR, kind="ExternalInput").ap()
    w1bhl_d = nc.dram_tensor("w1bhl", [64, 256], FPR, kind="ExternalInput").ap()
    w1cl_d = nc.dram_tensor("w1cl", [128, 256], FPR, kind="ExternalInput").ap()
    actc_d = nc.dram_tensor("actc", [128, 16], FP, kind="ExternalInput").ap()
    grub_d = nc.dram_tensor("grub", [1, 512], FPR, kind="ExternalInput").ap()
    ones_d = nc.dram_tensor("ones", [1, 128], FPR, kind="ExternalInput").ap()
    oo_d = nc.dram_tensor("oo", [BC, N, C], FP, kind="ExternalOutput").ap()
    oh_d = nc.dram_tensor("oh", [BC, N, C], FP, kind="ExternalOutput").ap()

    # tiled views: node dim -> (tile, partition)
    x_v = x_d.rearrange("b (k p) c -> b p k c", p=128)     # [BC,128,KT,64]
    h_v = h_d.rearrange("b (k p) c -> b p k c", p=128)
    hg_v = hg_d.rearrange("b (k p) c -> b p k c", p=128)
    xlo_v = xlo_d.rearrange("b (k p) c -> b p k c", p=128)
    hlo_v = hlo_d.rearrange("b (k p) c -> b p k c", p=128)
    adjt_v = adjt_d.rearrange("(k p) n -> p k n", p=128)   # [128,KT,1024]
    oo_v = oo_d.rearrange("b (k p) c -> b p k c", p=128)
    oh_v = oh_d.rearrange("b (k p) c -> b p k c", p=128)

    with tile.TileContext(nc) as tc:
        with (
            tc.tile_pool(name="const", bufs=1) as const_pool,
            tc.tile_pool(name="xh", bufs=3) as xh_pool,
            tc.tile_pool(name="a1", bufs=2) as a1_pool,
            tc.tile_pool(name="pp", bufs=2) as p_pool,
            tc.tile_pool(name="t1", bufs=2) as t1_pool,
            tc.tile_pool(name="gru", bufs=2) as gru_pool,
            tc.tile_pool(name="ps_mm13", bufs=2, space="PSUM") as ps13_pool,
            tc.tile_pool(name="ps_mm2", bufs=1, space="PSUM") as ps2_pool,
            tc.tile_pool(name="ps_mm4", bufs=2, space="PSUM") as ps4_pool,
        ):
            # ---- constants ----
            adjt_sb = const_pool.tile([128, KT, N], FPR)
            for nh2 in range(2):
                for kt in range(KT):
                    nc.sync.dma_start(out=adjt_sb[:, kt, nh2 * 512:(nh2 + 1) * 512],
                                      in_=adjt_v[:, kt, nh2 * 512:(nh2 + 1) * 512])
            w0_sb = const_pool.tile([128, CG], FPR)
            nc.sync.dma_start(out=w0_sb, in_=w0_d)
            w0l_sb = const_pool.tile([128, CG], FPR)
            nc.sync.dma_start(out=w0l_sb, in_=w0l_d)
            w1a_sb = const_pool.tile([128, 256], FPR)
            nc.sync.dma_start(out=w1a_sb, in_=w1a_d)
            w1bi_sb = const_pool.tile([64, 256], FPR)
            nc.sync.dma_start(out=w1bi_sb, in_=w1bi_d)
            w1bh_sb = const_pool.tile([64, 256], FPR)
            nc.sync.dma_start(out=w1bh_sb, in_=w1bh_d)
            w1c_sb = const_pool.tile([128, 256], FPR)
            nc.sync.dma_start(out=w1c_sb, in_=w1c_d)
            w1al_sb = const_pool.tile([128, 256], FPR)
            nc.sync.dma_start(out=w1al_sb, in_=w1al_d)
            w1bil_sb = const_pool.tile([64, 256], FPR)
            nc.sync.dma_start(out=w1bil_sb, in_=w1bil_d)
            w1bhl_sb = const_pool.tile([64, 256], FPR)
            nc.sync.dma_start(out=w1bhl_sb, in_=w1bhl_d)
            w1cl_sb = const_pool.tile([128, 256], FPR)
            nc.sync.dma_start(out=w1cl_sb, in_=w1cl_d)
            actc_sb = const_pool.tile([128, 16], FP)
            nc.sync.dma_start(out=actc_sb, in_=actc_d)
            grub_sb = const_pool.tile([1, 512], FPR)
            nc.sync.dma_start(out=grub_sb, in_=grub_d)
            ones_sb = const_pool.tile([1, 128], FPR)
            nc.sync.dma_start(out=ones_sb, in_=ones_d)

            t1 = None
            t1_of_pair = {}

            def do_mm4(pair):
                t1p = t1_of_pair.pop(pair)
                b0, b1 = 2 * pair, 2 * pair + 1
                hx_all = gru_pool.tile([128, KT, 2, C], FP, name=f"hx_all_{pair}", tag="hx_all")
                nc.sync.dma_start(out=hx_all[:, :, 0, :], in_=hg_v[b0])
                nc.sync.dma_start(out=hx_all[:, :, 1, :], in_=hg_v[b1])
                oo_all = gru_pool.tile([128, KT, 2, C], FP, name=f"oo_all_{pair}", tag="oo_all")
                oh_all = gru_pool.tile([128, KT, 2, C], FP, name=f"oh_all_{pair}", tag="oh_all")
                hx01 = gru_pool.tile([128, KT, 2, C], FP, name=f"hx01_{pair}", tag="hx01")
                nc.vector.tensor_scalar_mul(hx01, hx_all, 0.1)
                for nblk in range(KT):
                    ps4 = ps4_pool.tile([128, 512], FP, name=f"ps4_{pair}_{nblk}", tag="ps4")
                    nc.tensor.matmul(ps4, lhsT=(ones_sb), rhs=(grub_sb),
                                     start=True, stop=False)
                    for kt in range(KT):
                        nc.tensor.matmul(
                            ps4,
                            lhsT=(adjt_sb[:, kt, nblk * 128:(nblk + 1) * 128]),
                            rhs=(t1p[:, kt, :]),
                            start=False,
                            stop=(kt == KT - 1),
                        )
                    # ps4 cols per batch: [r~ 0:64 | i~ 64:128 | i_n 128:192 | h_n 192:256]
                    g = ps4.rearrange("p (two c) -> p two c", two=2)
                    ri = gru_pool.tile([128, 2, 128], FP, name=f"ri_{pair}_{nblk}", tag="ri")
                    nc.scalar.activation(ri, g[:, :, 0:128], AF.Sigmoid)
                    hx = hx_all[:, nblk]
                    t_ = gru_pool.tile([128, 2, C], FP, name=f"t_{pair}_{nblk}", tag="t_")
                    nc.vector.tensor_mul(t_, ri[:, :, 0:64], g[:, :, 192:256])
                    u = gru_pool.tile([128, 2, C], FP, name=f"u_{pair}_{nblk}", tag="u")
                    nc.vector.tensor_add(u, t_, g[:, :, 128:192])
                    ng = gru_pool.tile([128, 2, C], FP, name=f"ng_{pair}_{nblk}", tag="ng")
                    nc.scalar.activation(ng, u, AF.Tanh)
                    d = gru_pool.tile([128, 2, C], FP, name=f"d_{pair}_{nblk}", tag="d")
                    nc.vector.tensor_sub(d, hx, ng)
                    e = gru_pool.tile([128, 2, C], FP, name=f"e_{pair}_{nblk}", tag="e")
                    nc.vector.tensor_mul(e, d, ri[:, :, 64:128])
                    npre = oo_all[:, nblk]
                    nc.vector.tensor_add(npre, e, ng)
                    nh = oh_all[:, nblk]
                    nc.vector.scalar_tensor_tensor(nh, npre, 0.9, hx01[:, nblk],
                                                   op0=OP.mult, op1=OP.add)
                for i_, bb in ((0, b0), (1, b1)):
                    nc.sync.dma_start(out=oo_v[bb], in_=oo_all[:, :, i_, :])
                    nc.sync.dma_start(out=oh_v[bb], in_=oh_all[:, :, i_, :])

            for b in range(BC):
                half = b % 2
                if half == 0 and b >= 2:
                    do_mm4(b // 2 - 1)

                # ---- load x|h node-major ----
                xh = xh_pool.tile([128, KT, 256], FPR)
                nc.sync.dma_start(out=xh[:, :, 0:C], in_=x_v[b])
                nc.sync.dma_start(out=xh[:, :, C:128], in_=h_v[b])
                nc.sync.dma_start(out=xh[:, :, 128:128 + C], in_=xlo_v[b])
                nc.sync.dma_start(out=xh[:, :, 128 + C:256], in_=hlo_v[b])

                # ---- MM1: A1 = [x|h].T @ adjT  (channel-major adj conv) ----
                a1 = a1_pool.tile([128, N], FPR)
                for nch in range(2):
                    ps1 = ps13_pool.tile([128, 512], FP, tag="psA", name="ps1")
                    for part in range(2):
                        for kt in range(KT):
                            nc.tensor.matmul(
                                ps1,
                                lhsT=(xh[:, kt, part * 128:(part + 1) * 128]),
                                rhs=(adjt_sb[:, kt, nch * 512:(nch + 1) * 512]),
                                start=(part == 0 and kt == 0),
                                stop=(part == 1 and kt == KT - 1),
                            )
                    nc.vector.tensor_copy(a1[:, nch * 512:(nch + 1) * 512], ps1)

                # ---- MM2: S = A1 @ [Wi0|Wh0] + b0 ; P = prelu(S) (channel-major) ----
                p1 = p_pool.tile([128, N], FPR)    # i-branch cg 0:128
                p2i = p_pool.tile([64, N], FPR)    # i-branch cg 128:192
                p2h = p_pool.tile([64, N], FPR)    # h-branch cg 128:192
                p3 = p_pool.tile([128, N], FPR)    # h-branch cg 0:128
                for nch in range(2):
                    sl = slice(nch * 512, (nch + 1) * 512)
                    psi = ps2_pool.tile([128, 512], FP)
                    psh = ps2_pool.tile([128, 512], FP)
                    psmi = ps2_pool.tile([64, 512], FP)
                    psmh = ps2_pool.tile([64, 512], FP)
                    nc.tensor.matmul(psi, lhsT=(w0_sb[0:64, 0:128]),
                                     rhs=(a1[0:64, sl]), start=True, stop=False)
                    nc.tensor.matmul(psi, lhsT=(w0l_sb[0:64, 0:128]),
                                     rhs=(a1[0:64, sl]), start=False, stop=True)
                    nc.tensor.matmul(psh, lhsT=(w0_sb[64:128, 0:128]),
                                     rhs=(a1[64:128, sl]), start=True, stop=False)
                    nc.tensor.matmul(psh, lhsT=(w0l_sb[64:128, 0:128]),
                                     rhs=(a1[64:128, sl]), start=False, stop=True)
                    nc.tensor.matmul(psmi, lhsT=(w0_sb[0:64, 128:192]),
                                     rhs=(a1[0:64, sl]), start=True, stop=False)
                    nc.tensor.matmul(psmi, lhsT=(w0l_sb[0:64, 128:192]),
                                     rhs=(a1[0:64, sl]), start=False, stop=True)
                    nc.tensor.matmul(psmh, lhsT=(w0_sb[64:128, 128:192]),
                                     rhs=(a1[64:128, sl]), start=True, stop=False)
                    nc.tensor.matmul(psmh, lhsT=(w0l_sb[64:128, 128:192]),
                                     rhs=(a1[64:128, sl]), start=False, stop=True)
                    for ps, pt, pp, ci in ((psi, p1, 128, 0), (psh, p3, 128, 1),
                                           (psmi, p2i, 64, 2), (psmh, p2h, 64, 3)):
                        if prelu_native:
                            nc.scalar.activation(
                                pt[:, sl], ps, AF.Prelu,
                                bias=actc_sb[0:pp, ci:ci + 1],
                                alpha=actc_sb[0:pp, 4 + ci:5 + ci],
                            )
                        else:
                            rl = gru_pool.tile([128, 512], FP, tag="prelu_rl")
                            av = gru_pool.tile([128, 512], FP, tag="prelu_av")
                            nc.scalar.activation(rl[0:pp, :], ps, AF.Relu,
                                                 bias=actc_sb[0:pp, ci:ci + 1])
                            nc.scalar.activation(av[0:pp, :], ps, AF.Identity,
                                                 bias=actc_sb[0:pp, 8 + ci:9 + ci],
                                                 scale=actc_sb[0:pp, 4 + ci:5 + ci])
                            nc.vector.scalar_tensor_tensor(
                                pt[:, sl], rl[0:pp, :], actc_sb[0:pp, 12 + ci:13 + ci],
                                av[0:pp, :], op0=OP.mult, op1=OP.add)

                # ---- MM3: T1 = P @ W1cat (node-major, gate-merged 256 cols) ----
                if half == 0:
                    t1 = t1_pool.tile([128, KT, 512], FPR, name=f"t1_{b // 2}", tag="t1")
                    t1_of_pair[b // 2] = t1
                for nblk in range(KT):
                    nsl = slice(nblk * 128, (nblk + 1) * 128)
                    ps3 = ps13_pool.tile([128, 512], FP, tag="psA", name="ps3")[:, 0:256]
                    nc.tensor.matmul(ps3, lhsT=(p1[:, nsl]), rhs=(w1a_sb),
                                     start=True, stop=False)
                    nc.tensor.matmul(ps3, lhsT=(p1[:, nsl]), rhs=(w1al_sb),
                                     start=False, stop=False)
                    nc.tensor.matmul(ps3, lhsT=(p2i[:, nsl]), rhs=(w1bi_sb),
                                     start=False, stop=False)
                    nc.tensor.matmul(ps3, lhsT=(p2i[:, nsl]), rhs=(w1bil_sb),
                                     start=False, stop=False)
                    nc.tensor.matmul(ps3, lhsT=(p3[:, nsl]), rhs=(w1c_sb),
                                     start=False, stop=False)
                    nc.tensor.matmul(ps3, lhsT=(p3[:, nsl]), rhs=(w1cl_sb),
                                     start=False, stop=False)
                    nc.tensor.matmul(ps3, lhsT=(p2h[:, nsl]), rhs=(w1bh_sb),
                                     start=False, stop=False)
                    nc.tensor.matmul(ps3, lhsT=(p2h[:, nsl]), rhs=(w1bhl_sb),
                                     start=False, stop=True)
                    if nblk % 2 == 0:
                        nc.scalar.copy(t1[:, nblk, half * 256:(half + 1) * 256], ps3)
                    else:
                        nc.vector.tensor_copy(t1[:, nblk, half * 256:(half + 1) * 256], ps3)

            do_mm4(BC // 2 - 1)

    nc.compile()
    return nc


def round_fp32r(a):
    """Round fp32 to the PE's fp32r format: RNE to 11 mantissa bits."""
    u = np.ascontiguousarray(a, dtype=np.float32).view(np.uint32)
    r = u + (((u >> np.uint32(12)) & np.uint32(1)) + np.uint32(0x7FF))
    r &= np.uint32(0xFFFFF000)
    return r.view(np.float32)


def host_prep(inputs, adj, hidden, Wi0, bi0, Wi1, bi1, a_i,
              Wh0, bh0, Wh1, bh1, a_h, bias_r, bias_i, bias_n):
    f32 = lambda a: np.ascontiguousarray(np.asarray(a), dtype=np.float32)
    x = f32(inputs).reshape(B, N, C)
    h = f32(hidden)
    adjt = np.ascontiguousarray(f32(adj).T)
    Wi0, Wh0, Wi1, Wh1 = f32(Wi0), f32(Wh0), f32(Wi1), f32(Wh1)
    bi0, bh0, bi1, bh1 = f32(bi0), f32(bh0), f32(bi1), f32(bh1)
    bias_r, bias_i, bias_n = f32(bias_r), f32(bias_i), f32(bias_n)
    a_i = float(np.asarray(a_i)); a_h = float(np.asarray(a_h))

    w0 = np.concatenate([Wi0, Wh0], axis=0)                     # [128,192]
    z64 = np.zeros((128, 64), np.float32)
    w1a = np.concatenate([Wi1[0:128, 0:64], Wi1[0:128, 64:128],
                          Wi1[0:128, 128:192], z64], axis=1)    # i rows 0:128
    w1c = np.concatenate([Wh1[0:128, 0:64], Wh1[0:128, 64:128],
                          z64, Wh1[0:128, 128:192]], axis=1)    # h rows 0:128
    z = np.zeros((64, 64), np.float32)
    w1bi = np.concatenate([Wi1[128:192, 0:64], Wi1[128:192, 64:128],
                           Wi1[128:192, 128:192], z], axis=1)   # [64,256]
    w1bh = np.concatenate([Wh1[128:192, 0:64], Wh1[128:192, 64:128],
                           z, Wh1[128:192, 128:192]], axis=1)   # [64,256]

    pad64 = lambda v: np.concatenate([v, np.zeros(64, np.float32)])
    bias_cols = np.stack([bi0[0:128], bh0[0:128],
                          pad64(bi0[128:192]), pad64(bh0[128:192])], axis=1)
    alpha_cols = np.stack([np.full(128, a_i, np.float32),
                           np.full(128, a_h, np.float32),
                           np.full(128, a_i, np.float32),
                           np.full(128, a_h, np.float32)], axis=1)
    actc = np.concatenate([bias_cols, alpha_cols,
                           alpha_cols * bias_cols, 1.0 - alpha_cols],
                          axis=1).astype(np.float32)            # [128,16]

    gb = np.concatenate([bi1[0:64] + bh1[0:64] + bias_r,
                         bi1[64:128] + bh1[64:128] + bias_i,
                         bi1[128:192] + bias_n,
                         bh1[128:192]])
    grub = np.tile(gb, 2)[None, :].astype(np.float32)           # [1,512]

    def split_r(a):
        hi = round_fp32r(a)
        lo = round_fp32r(a - hi)
        return hi, lo

    w0_h, w0_l = split_r(w0)
    w1a_h, w1a_l = split_r(w1a)
    w1bi_h, w1bi_l = split_r(w1bi)
    w1bh_h, w1bh_l = split_r(w1bh)
    w1c_h, w1c_l = split_r(w1c)
    shared = dict(adjt=round_fp32r(adjt), w0=w0_h, w0l=w0_l,
                  w1a=w1a_h, w1al=w1a_l, w1bi=w1bi_h, w1bil=w1bi_l,
                  w1bh=w1bh_h, w1bhl=w1bh_l,
                  w1c=w1c_h, w1cl=w1c_l, actc=actc, grub=round_fp32r(grub),
                  ones=np.ones((1, 128), np.float32))
    xr, xlo = split_r(x)
    hr, hlo = split_r(h)
    in_maps = []
    for i in range(NCORES):
        m = dict(shared)
        sl = slice(i * BC, (i + 1) * BC)
        m["x"] = np.ascontiguousarray(xr[sl])
        m["h"] = np.ascontiguousarray(hr[sl])
        m["xlo"] = np.ascontiguousarray(xlo[sl])
        m["hlo"] = np.ascontiguousarray(hlo[sl])
        m["hg"] = np.ascontiguousarray(h[sl])
        in_maps.append(m)
    return in_maps


_CACHE = {}


def get_nc(prelu_native=True):
    key = prelu_native
    if key not in _CACHE:
        _CACHE[key] = build(prelu_native=prelu_native)
    return _CACHE[key]


def run(in_maps, trace=False, prelu_native=True, **kw):
    nc = get_nc(prelu_native)
    return run_bass_kernel_spmd(nc, in_maps, core_ids=list(range(NCORES)),
                                trace=trace, **kw)


def assemble(results):
    oo = np.concatenate([r["oo"] for r in results], axis=0)
    oh = np.concatenate([r["oh"] for r in results], axis=0)
    return oo.reshape(B, N, TOPK, F), oh


def kernel(**inputs):
    in_maps = host_prep(**inputs)
    res = run(in_maps, trace=False)
    return assemble(res.results)
